# revision 25
# baseline (speedup 1.0000x reference)
"""HQQ 4-bit quantized linear on 8 Trainium2 NeuronCores (Bass/Tile).

out[4096, 11008] = x[4096, 4096] @ dequant(W_q, scale, zero).T + bias

Column-parallel: core c owns g_rows [8c, 8c+8) of the 64-row nibble
matrix, i.e. the contiguous output slice o in [1376c, 1376c+1376).
Within a core, output col = r*172 + j (r = local g_row, j in [0,172)),
input i = k*128 + ii; group g = j*4096 + i.

Host staging (layout/bit-extract only - all arithmetic stays on device):
  - nibble of interest extracted to uint8 and pre-transposed to
    [k, ii, r, j] so dequant runs directly in the matmul layout
  - x pre-transposed/tiled to [tt, ii, k, tj] fp16
  - scale/zero transposed to [k, ii, j] fp16

Device per core:
  phase 1: per k-tile, ACT casts nib u8->fp16, then DVE computes
           W = (nib - zero_bcast) * scale_bcast in two all-fp16 passes
           (2x DVE mode, broadcast APs along r) into resident fp16
           WT[128, 32, 8, 172].  Interleaved with the dequant, the PE
           accumulates t-tiles 0,1 (all 3 o-banks) and t-tile 2's first
           two o-banks -- all 8 PSUM banks -- so the PE stays ~busy
           through phase 1.
  phase 2: per t-tile, one 1MB DMA of x.T tiles, then 32k x 3 o-bank
           back-to-back fp16 matmuls accumulating in PSUM; bias added
           during the PSUM->SBUF copy on DVE (bias replicated across
           partitions once by a broadcast DMA).
"""

import numpy as np
from contextlib import ExitStack

import concourse.bacc as bacc
import concourse.bass as bass
import concourse.mybir as mybir
import concourse.tile as tile
from concourse.bass_utils import run_bass_kernel_spmd

dt = mybir.dt

TOKENS, IN_F, OUT_F, GS = 4096, 4096, 11008, 64
J = 172                               # groups per (g_row, i) plane
NCORES = 8
RPC = GS // NCORES                    # 8 g_rows per core
O_C = RPC * J                         # 1376 output cols per core
NT = TOKENS // 128                    # 32 token tiles
NK = IN_F // 128                      # 32 contraction tiles
O_SPLITS = ((0, 512), (512, 512), (1024, 352))   # psum o-tiles (1 bank each)
KCH = 2                               # k-tiles dequantized per DVE pass

_CACHE = {}


def _build():
    nc = bacc.Bacc("TRN2", target_bir_lowering=False, debug=False,
                   num_devices=NCORES)

    NCH = NK // KCH
    nib_d = nc.dram_tensor("nib", [NCH, 128, KCH, RPC, J], dt.uint8,
                           kind="ExternalInput")
    sc_d = nc.dram_tensor("sc", [NCH, 128, KCH, J], dt.float16,
                          kind="ExternalInput")
    z_d = nc.dram_tensor("z", [NCH, 128, KCH, J], dt.float16,
                         kind="ExternalInput")
    xt_d = nc.dram_tensor("xt", [NT, 128, NK, 128], dt.float16,
                          kind="ExternalInput")
    b_d = nc.dram_tensor("bias", [1, O_C], dt.float32, kind="ExternalInput")
    o_d = nc.dram_tensor("out", [TOKENS, O_C], dt.float32,
                         kind="ExternalOutput")

    with ExitStack() as ctx:
        tc = ctx.enter_context(tile.TileContext(nc))
        const = ctx.enter_context(tc.tile_pool(name="const", bufs=1))
        ph1 = ctx.enter_context(tc.tile_pool(name="ph1", bufs=3))
        xp = ctx.enter_context(tc.tile_pool(name="xp", bufs=3))
        op = ctx.enter_context(tc.tile_pool(name="op", bufs=3))
        pacc = ctx.enter_context(
            tc.tile_pool(name="pacc", bufs=2, space=bass.MemorySpace.PSUM))
        pacc2 = ctx.enter_context(
            tc.tile_pool(name="pacc2", bufs=1, space=bass.MemorySpace.PSUM))

        WT = const.tile([128, NK, RPC, J], dt.float16)   # resident W.T

        # warm up DVE/ACT dispatch pipelines off the critical path
        warm = const.tile([1, 8], dt.float16)
        nc.vector.memset(warm[:], 0.0)
        warm2 = const.tile([1, 8], dt.float16)
        nc.scalar.copy(warm2[:], warm[:])

        def fetch_ch(c, split=False):
            nib = ph1.tile([128, KCH, RPC, J], dt.uint8, tag="nib",
                           name="nib")
            if split:
                # first k-tile + its scale/zero ahead of everything else
                nc.sync.dma_start(nib[:, 0], nib_d[c][:, 0])
                sct = ph1.tile([128, KCH, J], dt.float16, tag="sc",
                               name="sct")
                nc.sync.dma_start(sct[:], sc_d[c])
                zt = ph1.tile([128, KCH, J], dt.float16, tag="z", name="zt")
                nc.sync.dma_start(zt[:], z_d[c])
                for kk in range(1, KCH):
                    nc.sync.dma_start(nib[:, kk], nib_d[c][:, kk])
                return nib, sct, zt
            nc.sync.dma_start(nib[:], nib_d[c])
            sct = ph1.tile([128, KCH, J], dt.float16, tag="sc", name="sct")
            nc.sync.dma_start(sct[:], sc_d[c])
            zt = ph1.tile([128, KCH, J], dt.float16, tag="z", name="zt")
            nc.sync.dma_start(zt[:], z_d[c])
            return nib, sct, zt

        # first dequant chunks ahead of the big x.T slabs so the DVE can
        # start immediately
        pre = {0: fetch_ch(0, split=True), 1: fetch_ch(1)}

        xs_map = {}
        for t in range(3):
            xs = xp.tile([128, NK, 128], dt.float16, tag="xs", name=f"xs{t}")
            nc.sync.dma_start(xs[:], xt_d[t])
            xs_map[t] = xs

        accs = {}
        for t in (0, 1):
            accs[t] = [pacc.tile([128, on], dt.float32, tag=f"a{p}",
                                 name=f"a{p}")
                       for p, (ob, on) in enumerate(O_SPLITS)]
        x01 = pacc2.tile([128, 1024], dt.float32, name="x01")

        # ---- phase 1: dequantize into WT + early matmuls (8 psum banks)
        wk_flat = {}
        NCH = NK // KCH

        def early_mms(k):
            wk = WT[:, k].opt()            # flat [128, O_C] view
            wk_flat[k] = wk
            se = dict(start=(k == 0), stop=(k == NK - 1))
            for t in (0, 1):
                for p, (ob, on) in enumerate(O_SPLITS):
                    nc.tensor.matmul(accs[t][p][:], xs_map[t][:, k],
                                     wk[:, ob:ob + on], **se)
            nc.tensor.matmul(x01[:, 0:512], xs_map[2][:, k],
                             wk[:, 0:512], **se)
            nc.tensor.matmul(x01[:, 512:1024], xs_map[2][:, k],
                             wk[:, 512:1024], **se)

        # chunk 0 fast path: per-k dequant straight from u8 (no ACT cast
        # on the critical chain; DVE reads u8 at 1x but latency wins)
        nib0, sct0, zt0 = pre.pop(0)
        for kk in range(KCH):
            d1 = ph1.tile([128, RPC, J], dt.float16, tag="d1", name="d1")
            nc.vector.tensor_sub(
                d1[:], nib0[:, kk],
                zt0[:, kk].unsqueeze(1).broadcast_to((128, RPC, J)))
            nc.vector.tensor_mul(
                WT[:, kk], d1[:],
                sct0[:, kk].unsqueeze(1).broadcast_to((128, RPC, J)))
            early_mms(kk)

        for c in range(1, NCH):
            nib, sct, zt = pre.pop(c) if c in pre else fetch_ch(c)
            nibf = ph1.tile([128, KCH, RPC, J], dt.float16, tag="nibf",
                            name="nibf")
            nc.scalar.copy(nibf[:], nib[:])
            d = ph1.tile([128, KCH, RPC, J], dt.float16, tag="d", name="d")
            nc.vector.tensor_sub(
                d[:], nibf[:],
                zt[:].unsqueeze(2).broadcast_to((128, KCH, RPC, J)))
            nc.vector.tensor_mul(
                WT[:, c * KCH:(c + 1) * KCH], d[:],
                sct[:].unsqueeze(2).broadcast_to((128, KCH, RPC, J)))
            for k in range(c * KCH, (c + 1) * KCH):
                early_mms(k)

        biasf = const.tile([128, O_C], dt.float32)
        nc.sync.dma_start(biasf[:], b_d[:].to_broadcast((128, O_C)))

        def copy_out(t, psums, chunked=False):
            for p, (ap, ob, on) in enumerate(psums):
                if not chunked:
                    obp = op.tile([128, on], dt.float32, tag=f"ob{p}",
                                  name=f"ob{p}")
                    nc.vector.tensor_add(obp[:], ap, biasf[:, ob:ob + on])
                    nc.sync.dma_start(
                        o_d[t * 128:(t + 1) * 128, ob:ob + on], obp[:])
                    continue
                q = on // 4
                bounds = [(s * q, q if s < 3 else on - 3 * q)
                          for s in range(4)]
                for s, (cb, cn) in enumerate(bounds):
                    obp = op.tile([128, cn], dt.float32, tag=f"obc{p}_{s}",
                                  name=f"obc{p}")
                    nc.vector.tensor_add(
                        obp[:], ap[:, cb:cb + cn],
                        biasf[:, ob + cb:ob + cb + cn])
                    nc.sync.dma_start(
                        o_d[t * 128:(t + 1) * 128, ob + cb:ob + cb + cn],
                        obp[:])

        for t in (0, 1):
            copy_out(t, [(accs[t][p][:], ob, on)
                         for p, (ob, on) in enumerate(O_SPLITS)])

        # t=2: finish its third o-bank, then copy out
        a2t2 = pacc.tile([128, 352], dt.float32, tag="a2", name="a2")
        for k in range(NK):
            nc.tensor.matmul(a2t2[:], xs_map[2][:, k],
                             wk_flat[k][:, 1024:1376],
                             start=(k == 0), stop=(k == NK - 1))
        copy_out(2, [(x01[:, 0:512], 0, 512),
                     (x01[:, 512:1024], 512, 512),
                     (a2t2[:], 1024, 352)])

        # ---- phase 2: remaining t-tiles, dense matmul stream ----
        for t in range(3, NT):
            xs = xp.tile([128, NK, 128], dt.float16, tag="xs", name="xs")
            nc.sync.dma_start(xs[:], xt_d[t])
            acc = [pacc.tile([128, on], dt.float32, tag=f"a{p}",
                             name=f"a{p}")
                   for p, (ob, on) in enumerate(O_SPLITS)]
            for k in range(NK):
                wk = wk_flat[k]
                for p, (ob, on) in enumerate(O_SPLITS):
                    nc.tensor.matmul(
                        acc[p][:], xs[:, k], wk[:, ob:ob + on],
                        start=(k == 0), stop=(k == NK - 1))
            copy_out(t, [(acc[p][:], ob, on)
                         for p, (ob, on) in enumerate(O_SPLITS)],
                     chunked=(t == NT - 1))

    nc.compile()
    return nc


def get_nc():
    if "nc" not in _CACHE:
        _CACHE["nc"] = _build()
    return _CACHE["nc"]


def make_in_maps(x, W_q, scale, zero, bias):
    x = np.ascontiguousarray(x, dtype=np.float32)
    W_q = np.ascontiguousarray(W_q, dtype=np.int32)
    bias = np.ascontiguousarray(bias, dtype=np.float32)

    # x.T tiled: [tt, ii, k, tj] fp16
    xt = np.ascontiguousarray(
        x.T.reshape(NK, 128, NT, 128).transpose(2, 1, 0, 3)
    ).astype(np.float16)
    # scale/zero: [172, 4096] -> [c, ii, kk, j] fp16 (KCH k-tiles per chunk)
    NCH = NK // KCH
    s2 = (scale.reshape(J, IN_F).T.reshape(NCH, KCH, 128, J)
          .transpose(0, 2, 1, 3).astype(np.float16))
    z2 = (zero.reshape(J, IN_F).T.reshape(NCH, KCH, 128, J)
          .transpose(0, 2, 1, 3).astype(np.float16))
    s2 = np.ascontiguousarray(s2)
    z2 = np.ascontiguousarray(z2)

    in_maps = []
    for c in range(NCORES):
        r0 = RPC * (c % 4)
        q = W_q[r0:r0 + RPC]                       # [8, 704512]
        nib = ((q >> 4) if c < 4 else q) & 0xF     # hi rows for cores 0-3
        nib = nib.astype(np.uint8).reshape(RPC, J, IN_F)
        nib = (nib.transpose(2, 0, 1).reshape(NCH, KCH, 128, RPC, J)
               .transpose(0, 2, 1, 3, 4))
        nib = np.ascontiguousarray(nib)
        in_maps.append({
            "nib": nib,
            "sc": s2,
            "z": z2,
            "xt": xt,
            "bias": bias[c * O_C:(c + 1) * O_C].reshape(1, O_C),
        })
    return in_maps


def kernel(x, W_q, scale, zero, bias):
    nc = get_nc()
    in_maps = make_in_maps(x, W_q, scale, zero, bias)
    res = run_bass_kernel_spmd(nc, in_maps, list(range(NCORES)))
    return np.concatenate(
        [res.results[c]["out"] for c in range(NCORES)], axis=1)


# revision 29
# speedup vs baseline: 1.0037x; 1.0037x over previous
"""HQQ 4-bit quantized linear on 8 Trainium2 NeuronCores (Bass/Tile).

out[4096, 11008] = x[4096, 4096] @ dequant(W_q, scale, zero).T + bias

Column-parallel: core c owns g_rows [8c, 8c+8) of the 64-row nibble
matrix, i.e. the contiguous output slice o in [1376c, 1376c+1376).
Within a core, output col = r*172 + j (r = local g_row, j in [0,172)),
input i = k*128 + ii; group g = j*4096 + i.

Host staging (layout/bit-extract only - all arithmetic stays on device):
  - nibble of interest extracted to uint8 and pre-transposed to
    [k, ii, r, j] so dequant runs directly in the matmul layout
  - x pre-transposed/tiled to [tt, ii, k, tj] fp16
  - scale/zero transposed to [k, ii, j] fp16

Device per core:
  phase 1: per k-tile, ACT casts nib u8->fp16, then DVE computes
           W = (nib - zero_bcast) * scale_bcast in two all-fp16 passes
           (2x DVE mode, broadcast APs along r) into resident fp16
           WT[128, 32, 8, 172].  Interleaved with the dequant, the PE
           accumulates t-tiles 0,1 (all 3 o-banks) and t-tile 2's first
           two o-banks -- all 8 PSUM banks -- so the PE stays ~busy
           through phase 1.
  phase 2: per t-tile, one 1MB DMA of x.T tiles, then 32k x 3 o-bank
           back-to-back fp16 matmuls accumulating in PSUM; bias added
           during the PSUM->SBUF copy on DVE (bias replicated across
           partitions once by a broadcast DMA).
"""

import numpy as np
from contextlib import ExitStack

import concourse.bacc as bacc
import concourse.bass as bass
import concourse.mybir as mybir
import concourse.tile as tile
from concourse.bass_utils import run_bass_kernel_spmd

dt = mybir.dt

TOKENS, IN_F, OUT_F, GS = 4096, 4096, 11008, 64
J = 172                               # groups per (g_row, i) plane
NCORES = 8
RPC = GS // NCORES                    # 8 g_rows per core
O_C = RPC * J                         # 1376 output cols per core
NT = TOKENS // 128                    # 32 token tiles
NK = IN_F // 128                      # 32 contraction tiles
O_SPLITS = ((0, 512), (512, 512), (1024, 352))   # psum o-tiles (1 bank each)
KCH = 2                               # k-tiles dequantized per DVE pass

_CACHE = {}


def _build():
    nc = bacc.Bacc("TRN2", target_bir_lowering=False, debug=False,
                   num_devices=NCORES)

    NCH = NK // KCH
    nib_d = nc.dram_tensor("nib", [NCH, 128, KCH, RPC, J], dt.uint8,
                           kind="ExternalInput")
    sc_d = nc.dram_tensor("sc", [NCH, 128, KCH, J], dt.float16,
                          kind="ExternalInput")
    z_d = nc.dram_tensor("z", [NCH, 128, KCH, J], dt.float16,
                         kind="ExternalInput")
    xt_d = nc.dram_tensor("xt", [NT, 128, NK, 128], dt.float16,
                          kind="ExternalInput")
    b_d = nc.dram_tensor("bias", [1, O_C], dt.float32, kind="ExternalInput")
    o_d = nc.dram_tensor("out", [TOKENS, O_C], dt.float32,
                         kind="ExternalOutput")

    with ExitStack() as ctx:
        tc = ctx.enter_context(tile.TileContext(nc))
        const = ctx.enter_context(tc.tile_pool(name="const", bufs=1))
        ph1 = ctx.enter_context(tc.tile_pool(name="ph1", bufs=3))
        xp = ctx.enter_context(tc.tile_pool(name="xp", bufs=3))
        op = ctx.enter_context(tc.tile_pool(name="op", bufs=3))
        pacc = ctx.enter_context(
            tc.tile_pool(name="pacc", bufs=2, space=bass.MemorySpace.PSUM))
        pacc2 = ctx.enter_context(
            tc.tile_pool(name="pacc2", bufs=1, space=bass.MemorySpace.PSUM))

        WT = const.tile([128, NK, RPC, J], dt.float16)   # resident W.T

        def fetch_ch(c, split=False):
            nib = ph1.tile([128, KCH, RPC, J], dt.uint8, tag="nib",
                           name="nib")
            if split:
                # first k-tile in its own DMA so dequant starts sooner
                for kk in range(KCH):
                    nc.sync.dma_start(nib[:, kk], nib_d[c][:, kk])
            else:
                nc.sync.dma_start(nib[:], nib_d[c])
            sct = ph1.tile([128, KCH, J], dt.float16, tag="sc", name="sct")
            nc.sync.dma_start(sct[:], sc_d[c])
            zt = ph1.tile([128, KCH, J], dt.float16, tag="z", name="zt")
            nc.sync.dma_start(zt[:], z_d[c])
            return nib, sct, zt

        # first dequant chunks ahead of the big x.T slabs so the DVE can
        # start immediately
        pre = {0: fetch_ch(0, split=True), 1: fetch_ch(1)}

        xs_map = {}
        for t in range(3):
            xs = xp.tile([128, NK, 128], dt.float16, tag="xs", name=f"xs{t}")
            nc.sync.dma_start(xs[:], xt_d[t])
            xs_map[t] = xs

        biasf = const.tile([128, O_C], dt.float32)
        nc.sync.dma_start(biasf[:], b_d[:].to_broadcast((128, O_C)))

        accs = {}
        for t in (0, 1):
            accs[t] = [pacc.tile([128, on], dt.float32, tag=f"a{p}",
                                 name=f"a{p}")
                       for p, (ob, on) in enumerate(O_SPLITS)]
        x01 = pacc2.tile([128, 1024], dt.float32, name="x01")

        # ---- phase 1: dequantize into WT + early matmuls (8 psum banks)
        wk_flat = {}
        NCH = NK // KCH

        def early_mms(k):
            wk = WT[:, k].opt()            # flat [128, O_C] view
            wk_flat[k] = wk
            se = dict(start=(k == 0), stop=(k == NK - 1))
            for t in (0, 1):
                for p, (ob, on) in enumerate(O_SPLITS):
                    nc.tensor.matmul(accs[t][p][:], xs_map[t][:, k],
                                     wk[:, ob:ob + on], **se)
            nc.tensor.matmul(x01[:, 0:512], xs_map[2][:, k],
                             wk[:, 0:512], **se)
            nc.tensor.matmul(x01[:, 512:1024], xs_map[2][:, k],
                             wk[:, 512:1024], **se)

        # chunk 0 fast path: per-k dequant straight from u8 (no ACT cast
        # on the critical chain; DVE reads u8 at 1x but latency wins)
        nib0, sct0, zt0 = pre.pop(0)
        for kk in range(KCH):
            d1 = ph1.tile([128, RPC, J], dt.float16, tag="d1", name="d1")
            nc.vector.tensor_sub(
                d1[:], nib0[:, kk],
                zt0[:, kk].unsqueeze(1).broadcast_to((128, RPC, J)))
            nc.vector.tensor_mul(
                WT[:, kk], d1[:],
                sct0[:, kk].unsqueeze(1).broadcast_to((128, RPC, J)))
            early_mms(kk)

        for c in range(1, NCH):
            nib, sct, zt = pre.pop(c) if c in pre else fetch_ch(c)
            nibf = ph1.tile([128, KCH, RPC, J], dt.float16, tag="nibf",
                            name="nibf")
            nc.scalar.copy(nibf[:], nib[:])
            d = ph1.tile([128, KCH, RPC, J], dt.float16, tag="d", name="d")
            nc.vector.tensor_sub(
                d[:], nibf[:],
                zt[:].unsqueeze(2).broadcast_to((128, KCH, RPC, J)))
            nc.vector.tensor_mul(
                WT[:, c * KCH:(c + 1) * KCH], d[:],
                sct[:].unsqueeze(2).broadcast_to((128, KCH, RPC, J)))
            for k in range(c * KCH, (c + 1) * KCH):
                early_mms(k)

        def copy_out(t, psums, chunked=False):
            for p, (ap, ob, on) in enumerate(psums):
                if not chunked:
                    obp = op.tile([128, on], dt.float32, tag=f"ob{p}",
                                  name=f"ob{p}")
                    nc.vector.tensor_add(obp[:], ap, biasf[:, ob:ob + on])
                    nc.sync.dma_start(
                        o_d[t * 128:(t + 1) * 128, ob:ob + on], obp[:])
                    continue
                h = on // 2
                for s, (cb, cn) in enumerate(((0, h), (h, on - h))):
                    obp = op.tile([128, cn], dt.float32, tag=f"obc{p}_{s}",
                                  name=f"obc{p}")
                    nc.vector.tensor_add(
                        obp[:], ap[:, cb:cb + cn],
                        biasf[:, ob + cb:ob + cb + cn])
                    nc.sync.dma_start(
                        o_d[t * 128:(t + 1) * 128, ob + cb:ob + cb + cn],
                        obp[:])

        for t in (0, 1):
            copy_out(t, [(accs[t][p][:], ob, on)
                         for p, (ob, on) in enumerate(O_SPLITS)])

        # t=2: finish its third o-bank, then copy out
        a2t2 = pacc.tile([128, 352], dt.float32, tag="a2", name="a2")
        for k in range(NK):
            nc.tensor.matmul(a2t2[:], xs_map[2][:, k],
                             wk_flat[k][:, 1024:1376],
                             start=(k == 0), stop=(k == NK - 1))
        copy_out(2, [(x01[:, 0:512], 0, 512),
                     (x01[:, 512:1024], 512, 512),
                     (a2t2[:], 1024, 352)])

        # ---- phase 2: remaining t-tiles, dense matmul stream ----
        for t in range(3, NT):
            xs = xp.tile([128, NK, 128], dt.float16, tag="xs", name="xs")
            nc.sync.dma_start(xs[:], xt_d[t])
            acc = [pacc.tile([128, on], dt.float32, tag=f"a{p}",
                             name=f"a{p}")
                   for p, (ob, on) in enumerate(O_SPLITS)]
            for k in range(NK):
                wk = wk_flat[k]
                for p, (ob, on) in enumerate(O_SPLITS):
                    nc.tensor.matmul(
                        acc[p][:], xs[:, k], wk[:, ob:ob + on],
                        start=(k == 0), stop=(k == NK - 1))
            copy_out(t, [(acc[p][:], ob, on)
                         for p, (ob, on) in enumerate(O_SPLITS)],
                     chunked=(t == NT - 1))

    nc.compile()
    return nc


def get_nc():
    if "nc" not in _CACHE:
        _CACHE["nc"] = _build()
    return _CACHE["nc"]


def make_in_maps(x, W_q, scale, zero, bias):
    x = np.ascontiguousarray(x, dtype=np.float32)
    W_q = np.ascontiguousarray(W_q, dtype=np.int32)
    bias = np.ascontiguousarray(bias, dtype=np.float32)

    # x.T tiled: [tt, ii, k, tj] fp16
    xt = np.ascontiguousarray(
        x.T.reshape(NK, 128, NT, 128).transpose(2, 1, 0, 3)
    ).astype(np.float16)
    # scale/zero: [172, 4096] -> [c, ii, kk, j] fp16 (KCH k-tiles per chunk)
    NCH = NK // KCH
    s2 = (scale.reshape(J, IN_F).T.reshape(NCH, KCH, 128, J)
          .transpose(0, 2, 1, 3).astype(np.float16))
    z2 = (zero.reshape(J, IN_F).T.reshape(NCH, KCH, 128, J)
          .transpose(0, 2, 1, 3).astype(np.float16))
    s2 = np.ascontiguousarray(s2)
    z2 = np.ascontiguousarray(z2)

    in_maps = []
    for c in range(NCORES):
        r0 = RPC * (c % 4)
        q = W_q[r0:r0 + RPC]                       # [8, 704512]
        nib = ((q >> 4) if c < 4 else q) & 0xF     # hi rows for cores 0-3
        nib = nib.astype(np.uint8).reshape(RPC, J, IN_F)
        nib = (nib.transpose(2, 0, 1).reshape(NCH, KCH, 128, RPC, J)
               .transpose(0, 2, 1, 3, 4))
        nib = np.ascontiguousarray(nib)
        in_maps.append({
            "nib": nib,
            "sc": s2,
            "z": z2,
            "xt": xt,
            "bias": bias[c * O_C:(c + 1) * O_C].reshape(1, O_C),
        })
    return in_maps


def kernel(x, W_q, scale, zero, bias):
    nc = get_nc()
    in_maps = make_in_maps(x, W_q, scale, zero, bias)
    res = run_bass_kernel_spmd(nc, in_maps, list(range(NCORES)))
    return np.concatenate(
        [res.results[c]["out"] for c in range(NCORES)], axis=1)
